# revision 1
# baseline (speedup 1.0000x reference)
"""Multi-head self-attention (causal) Trainium2 Bass kernel, 8-core SPMD.

Sharding: 8 cores = 2 batches x 4 head-groups (3 heads each).
Each core computes, for its (batch, head-group):
  - Q^T, K^T, V projections from a host-pretransposed x^T (bf16)
  - causal attention with scores kept transposed (S^T[k,q]) so no on-device
    transposes are needed; softmax denominator comes free via a ones-column
    appended to V
  - its 3 heads' slice of the output projection (partial sum over d)
Host gathers: out[b] = sum of 4 group partials + (b_proj + b_v @ W_proj).
b_k is dropped (softmax row-shift invariance), b_v folded into host bias.

Layout notes: heads 0/1 are packed into partition halves 0:64 / 64:128 so
their score matmuls land in different PE row groups (concurrent on HW) and
the projection contracts over 128 partitions in one matmul. Cross-partition
moves (head-2 Q^T, head-1 attn^T) are done with small SBUF->SBUF DMAs,
which are the only engines that can re-partition.
"""

import numpy as np
import ml_dtypes

S = 2048          # sequence length
D = 768           # model dim
HD = 64           # head dim
HPC = 3           # heads per core
NCORES = 8
P = 128           # partitions
CT = D // P       # 6 contraction tiles over model dim
KT = S // P       # 16 key tiles
QC = 512          # query chunk (PSUM bank width in fp32)
NQC = S // QC     # 4 query chunks

_BF = ml_dtypes.bfloat16

_cache = {}


def _build_nc():
    import concourse.bass as bass
    import concourse.mybir as mybir
    import concourse.tile as tile
    from concourse import bacc
    from contextlib import ExitStack

    bf = mybir.dt.bfloat16
    f32 = mybir.dt.float32

    nc = bacc.Bacc()
    xT = nc.declare_dram_parameter("xT", [D, S], bf, isOutput=False)
    # 3 lhsT slots per c-tile: 0=[Wk0|Wk1] 1=[Wq0|Wq1] 2=[Wk2|Wq2]
    w_qk = nc.declare_dram_parameter("w_qk", [D, 3, P], bf, isOutput=False)
    w_v = nc.declare_dram_parameter("w_v", [D, HPC * HD], bf, isOutput=False)
    # col 0: [bq_h0 | bq_h1]; col 1: rows 64:128 = bq_h2
    bq = nc.declare_dram_parameter("bq", [P, 2], f32, isOutput=False)
    # rows: W_proj rows of h0, h1, h2 stacked
    w_p = nc.declare_dram_parameter("w_p", [HPC * HD, D], bf, isOutput=False)
    mask = nc.declare_dram_parameter("mask", [P, P], bf, isOutput=False)
    out_p = nc.declare_dram_parameter("out_p", [S, D], f32, isOutput=True)

    Exp = mybir.ActivationFunctionType.Exp

    with tile.TileContext(nc) as tc, ExitStack() as ctx:
        singles = ctx.enter_context(tc.tile_pool(name="singles", bufs=1))
        pmm = ctx.enter_context(tc.tile_pool(name="pmm", bufs=2, space="PSUM"))
        # scores + projection share this pool's 3 banks (disjoint phases)
        ps_pool = ctx.enter_context(tc.tile_pool(name="ps", bufs=3, space="PSUM"))
        po_pool = ctx.enter_context(tc.tile_pool(name="po", bufs=3, space="PSUM"))
        pt_pool = ctx.enter_context(tc.tile_pool(name="pt", bufs=12))
        norm_pool = ctx.enter_context(tc.tile_pool(name="norm", bufs=3))
        outs_pool = ctx.enter_context(tc.tile_pool(name="outs", bufs=3))

        # ---- persistent SBUF ----
        # DMA issue costs ~0.5us on the issuing engine: keep the count low,
        # small critical inputs first, and bulk loads split SP/gpsimd.
        xT_s = singles.tile([P, CT, S], bf)
        wqk_s = singles.tile([P, CT, 3, P], bf)
        wv_s = singles.tile([P, CT, HPC * HD], bf)
        bq_s = singles.tile([P, 2], f32)
        mask_s = singles.tile([P, P], bf)
        wpa_s = singles.tile([P, D], bf)
        wpb_s = singles.tile([HD, D], bf)
        wpb1_s = singles.tile([HD, D], bf)  # h1 proj rows at base partition 0
        xt_r = xT.rearrange("(t p) q -> p t q", p=P)
        nc.gpsimd.dma_start(out=xT_s[:, 0:1, 0:QC], in_=xt_r[:, 0:1, 0:QC])
        nc.gpsimd.dma_start(out=xT_s[:, 1:3, 0:QC], in_=xt_r[:, 1:3, 0:QC])
        nc.scalar.dma_start(out=xT_s[:, 3:CT, 0:QC], in_=xt_r[:, 3:CT, 0:QC])
        wqk_r = w_qk.rearrange("(t p) s m -> p t s m", p=P)
        nc.sync.dma_start(out=wqk_s[:, 0:1], in_=wqk_r[:, 0:1])
        nc.sync.dma_start(out=wqk_s[:, 1:3], in_=wqk_r[:, 1:3])
        nc.sync.dma_start(out=wqk_s[:, 3:CT], in_=wqk_r[:, 3:CT])
        nc.gpsimd.dma_start(out=bq_s, in_=bq[:])
        nc.gpsimd.dma_start(out=mask_s, in_=mask[:])
        wv_r = w_v.rearrange("(t p) m -> p t m", p=P)
        nc.gpsimd.dma_start(out=wv_s[:, 0:3], in_=wv_r[:, 0:3])
        nc.gpsimd.dma_start(out=wv_s[:, 3:CT], in_=wv_r[:, 3:CT])
        for qc in range(1, NQC):
            nc.gpsimd.dma_start(out=xT_s[:, :, qc * QC:(qc + 1) * QC],
                                in_=xt_r[:, :, qc * QC:(qc + 1) * QC])
        nc.gpsimd.dma_start(out=wpa_s, in_=w_p[0:P, :])
        nc.gpsimd.dma_start(out=wpb1_s, in_=w_p[HD:P, :])
        nc.gpsimd.dma_start(out=wpb_s, in_=w_p[P:P + HD, :])

        # Q^T/K^T: slot 0 holds head0 (parts 0:64) + head1 (parts 64:128),
        # slot 1 holds head2 (parts 0:64; qt slot1 filled via repartition DMA).
        qt_s = singles.tile([P, 2, S], bf)
        kt_s = singles.tile([P, 2, S], bf)
        # V with a ones column appended per head (softmax denominator trick)
        v_s = singles.tile([P, KT, HPC, HD + 1], bf)
        nc.gpsimd.memset(v_s[:, :, :, HD:HD + 1], 1.0)
        # attn^T: h0 at parts 0:64, h1 at parts 64:128 (via DMA), h2 separate
        attn01_s = singles.tile([P, S], bf)
        attn2_s = singles.tile([HD, S], bf)
        attn1b_s = singles.tile([HD, QC], bf)  # last chunk's h1, un-repartitioned

        def proj_fillers(c, use_act=False, three_way=False):
            # output projection of chunk c, one filler per q-tile.
            # three_way (last chunk): one matmul per head so the projection
            # starts as soon as head 0 is normalized, skipping the h1
            # repartition DMA on the critical tail.
            def one(t):
                def f(dep=None):
                    ob = outs_pool.tile([P, D], f32, tag="ob", name="ob")
                    for e0, en in ((0, 512), (512, 256)):
                        pp = pmm.tile([P, QC], f32, tag="mm", name="pp")
                        if three_way:
                            tb = t * P - (NQC - 1) * QC
                            nc.tensor.matmul(pp[:, 0:en],
                                             lhsT=attn01_s[0:HD, t * P:(t + 1) * P],
                                             rhs=wpa_s[0:HD, e0:e0 + en],
                                             start=True, stop=False)
                            nc.tensor.matmul(pp[:, 0:en],
                                             lhsT=attn1b_s[:, tb:tb + P],
                                             rhs=wpb1_s[:, e0:e0 + en],
                                             start=False, stop=False)
                            mm = None
                        else:
                            mm = nc.tensor.matmul(pp[:, 0:en],
                                                  lhsT=attn01_s[:, t * P:(t + 1) * P],
                                                  rhs=wpa_s[:, e0:e0 + en],
                                                  start=True, stop=False)
                        if dep is not None and mm is not None:
                            tile.add_dep_helper(mm.ins, dep.ins, sync=False,
                                                reason="hold filler past last AV")
                            dep = None
                        nc.tensor.matmul(pp[:, 0:en],
                                         lhsT=attn2_s[:, t * P:(t + 1) * P],
                                         rhs=wpb_s[:, e0:e0 + en],
                                         start=False, stop=True)
                        if use_act:  # tail: ACT is idle, DVE still normalizing
                            nc.scalar.copy(out=ob[:, e0:e0 + en], in_=pp[:, 0:en])
                            nc.sync.dma_start(
                                out=out_p[t * P:(t + 1) * P, e0:e0 + en],
                                in_=ob[:, e0:e0 + en])
                        else:
                            nc.vector.tensor_copy(out=ob[:, e0:e0 + en], in_=pp[:, 0:en])
                    if not use_act:
                        nc.sync.dma_start(out=out_p[t * P:(t + 1) * P, :], in_=ob)
                return f
            return [one(t) for t in range(4 * c, 4 * c + 4)]

        def qkv_fillers(c):
            # Q^T/K^T/V projections for chunk c, as 7 PE filler groups
            qs = c * QC
            qsl = slice(qs, qs + QC)

            def g_kk():
                ps_kk = pmm.tile([P, QC], f32, tag="mm", name="ps_kk")
                for ct in range(CT):
                    nc.tensor.matmul(ps_kk, lhsT=wqk_s[:, ct, 0, :],
                                     rhs=xT_s[:, ct, qsl],
                                     start=(ct == 0), stop=(ct == CT - 1))
                nc.vector.tensor_copy(out=kt_s[:, 0, qsl], in_=ps_kk)

            def g_qq():
                ps_qq = pmm.tile([P, QC], f32, tag="mm", name="ps_qq")
                for ct in range(CT):
                    nc.tensor.matmul(ps_qq, lhsT=wqk_s[:, ct, 1, :],
                                     rhs=xT_s[:, ct, qsl],
                                     start=(ct == 0), stop=(ct == CT - 1))
                nc.vector.tensor_scalar_add(out=qt_s[:, 0, qsl], in0=ps_qq,
                                            scalar1=bq_s[:, 0:1])

            def g_kq2():
                ps_kq2 = pmm.tile([P, QC], f32, tag="mm", name="ps_kq2")
                for ct in range(CT):
                    nc.tensor.matmul(ps_kq2, lhsT=wqk_s[:, ct, 2, :],
                                     rhs=xT_s[:, ct, qsl],
                                     start=(ct == 0), stop=(ct == CT - 1))
                nc.vector.tensor_copy(out=kt_s[0:HD, 1, qsl], in_=ps_kq2[0:HD, :])
                # head2 Q lands in parts 64:128; bias-add, then repartition DMA
                q2st = norm_pool.tile([P, QC], bf, tag="q2st", name="q2st")
                nc.vector.tensor_scalar_add(out=q2st[HD:P, :], in0=ps_kq2[HD:P, :],
                                            scalar1=bq_s[HD:P, 1:2])
                nc.sync.dma_start(out=qt_s[0:HD, 1, qsl], in_=q2st[HD:P, :])

            def g_v(kt):
                def f():
                    ps_v = pmm.tile([P, QC], f32, tag="mm", name="ps_v")
                    for ct in range(CT):
                        nc.tensor.matmul(ps_v[:, 0:HPC * HD],
                                         lhsT=xT_s[:, ct, kt * P:(kt + 1) * P],
                                         rhs=wv_s[:, ct, :],
                                         start=(ct == 0), stop=(ct == CT - 1))
                    nc.vector.tensor_copy(
                        out=v_s[:, kt, :, 0:HD],
                        in_=ps_v[:, 0:HPC * HD].rearrange("p (h d) -> p h d", h=HPC))
                return f
            return [g_kk, g_qq, g_kq2] + [g_v(kt) for kt in range(4 * c, 4 * c + 4)]

        hsl = [slice(0, HD), slice(HD, P), slice(0, HD)]
        hslot = [0, 0, 1]

        for f in qkv_fillers(0):
            f()

        for c in range(NQC):
            qs = c * QC
            qsl = slice(qs, qs + QC)
            # fillers woven into this chunk's attention: next chunk's QKV,
            # then the previous chunk's projection
            fillers = []
            if c + 1 < NQC:
                fillers += qkv_fillers(c + 1)
            if c > 0:
                fillers += proj_fillers(c - 1)

            # ---- attention for q-chunk c (kt-major; h0/h1 scores can overlap
            # in different PE row groups) ----
            po = [po_pool.tile([P, QC], f32, tag="po", name=f"po{h}")
                  for h in range(HPC)]
            nkt = 4 * c + 4

            def emit_scores(kt):
                off = max(0, kt * P - qs)
                n = QC - off
                pss = []
                for h in range(HPC):
                    ps_s = ps_pool.tile([P, QC], f32, tag="ss", name="ps_s")
                    nc.tensor.matmul(ps_s[:, 0:n],
                                     lhsT=kt_s[hsl[h], hslot[h], kt * P:(kt + 1) * P],
                                     rhs=qt_s[hsl[h], hslot[h], qs + off:qs + QC],
                                     start=True, stop=True)
                    pss.append(ps_s)
                return pss

            def emit_avs(kt, pss):
                off = max(0, kt * P - qs)
                n = QC - off
                diag = kt * P >= qs
                mm = None
                for h in range(HPC):
                    pt = pt_pool.tile([P, QC], bf, tag="pt", name="pt")
                    nc.scalar.activation(out=pt[:, off:QC], in_=pss[h][:, 0:n],
                                         func=Exp, scale=0.125)
                    if diag:  # mask k>q inside the diagonal 128x128 block
                        nc.gpsimd.tensor_mul(out=pt[:, off:off + P],
                                             in0=pt[:, off:off + P], in1=mask_s)
                    mm = nc.tensor.matmul(po[h][0:HD + 1, off:QC],
                                          lhsT=v_s[:, kt, h, :], rhs=pt[:, off:QC],
                                          start=(kt == 0), stop=(kt == nkt - 1))
                return mm

            emitted = 0
            # last chunk: reserve two fillers to run after the final AV (they
            # bridge the normalize window), pace the rest into the kt loop
            n_defer = 0
            n_pace = len(fillers) - n_defer
            w0 = 1 if c + 1 < NQC else max(1, nkt - 3 * len(fillers))
            prev = emit_scores(0)
            for kt in range(1, nkt):
                cur = emit_scores(kt)
                emit_avs(kt - 1, prev)
                prev = cur
                if kt >= w0:
                    want = ((kt - w0 + 1) * n_pace) // max(1, nkt - w0)
                    while emitted < want:
                        fillers[emitted]()
                        emitted += 1
            last_av = emit_avs(nkt - 1, prev)
            while emitted < n_pace:
                fillers[emitted]()
                emitted += 1
            while emitted < len(fillers):
                fillers[emitted](dep=last_av)
                emitted += 1

            # normalize: recip of denom (DVE), broadcast (Pool), multiply (DVE)
            for h in range(HPC):
                recip = norm_pool.tile([HD + 1, QC], f32, tag="recip", name="recip")
                nc.vector.reciprocal(out=recip[HD:HD + 1, :], in_=po[h][HD:HD + 1, :])
                bcast = norm_pool.tile([HD, 1, QC], f32, tag="bcast", name="bcast")
                rs = recip[HD:HD + 1, :]
                rep = bass.AP(tensor=rs.tensor, offset=rs.offset,
                              ap=[list(rs.ap[0]), [0, HD], list(rs.ap[1])])
                nc.gpsimd.dma_start(out=bcast, in_=rep)
                if h == 0:
                    nc.vector.tensor_mul(out=attn01_s[0:HD, qsl], in0=po[h][0:HD, :],
                                         in1=bcast[:, 0, :])
                elif h == 1:
                    if c + 1 == NQC:
                        nc.vector.tensor_mul(out=attn1b_s, in0=po[h][0:HD, :],
                                             in1=bcast[:, 0, :])
                    else:
                        a1 = norm_pool.tile([HD, QC], bf, tag="a1", name="a1")
                        nc.vector.tensor_mul(out=a1, in0=po[h][0:HD, :],
                                             in1=bcast[:, 0, :])
                        nc.sync.dma_start(out=attn01_s[HD:P, qsl], in_=a1)
                else:
                    nc.vector.tensor_mul(out=attn2_s[:, qsl], in0=po[h][0:HD, :],
                                         in1=bcast[:, 0, :])

        for f in proj_fillers(NQC - 1, use_act=True, three_way=True):
            f()

    nc.compile()
    return nc


def _prep_inputs(x, W_qkv, b_qkv, W_proj):
    """Build the 8 per-core input maps (all bf16 except biases)."""
    in_maps = []
    for cid in range(NCORES):
        b, g = divmod(cid, 4)
        hs = [g * HPC + i for i in range(HPC)]  # global head ids

        def wslice(kind, h):  # kind 0=q 1=k 2=v
            return W_qkv[:, kind * D + h * HD:(kind * D + (h + 1) * HD)]

        xT = np.ascontiguousarray(x[b].T).astype(_BF)

        w_qk = np.zeros((D, 3, P), dtype=np.float32)
        w_qk[:, 0, 0:HD] = wslice(1, hs[0])
        w_qk[:, 0, HD:P] = wslice(1, hs[1])
        w_qk[:, 1, 0:HD] = wslice(0, hs[0])
        w_qk[:, 1, HD:P] = wslice(0, hs[1])
        w_qk[:, 2, 0:HD] = wslice(1, hs[2])
        w_qk[:, 2, HD:P] = wslice(0, hs[2])

        w_v = np.concatenate([wslice(2, h) for h in hs], axis=1)

        bq = np.zeros((P, 2), dtype=np.float32)
        bq[0:HD, 0] = b_qkv[hs[0] * HD:(hs[0] + 1) * HD]
        bq[HD:P, 0] = b_qkv[hs[1] * HD:(hs[1] + 1) * HD]
        bq[HD:P, 1] = b_qkv[hs[2] * HD:(hs[2] + 1) * HD]

        w_p = np.concatenate([W_proj[h * HD:(h + 1) * HD, :] for h in hs], axis=0)

        mask = np.triu(np.ones((P, P), dtype=np.float32))

        in_maps.append({
            "xT": xT,
            "w_qk": w_qk.astype(_BF),
            "w_v": w_v.astype(_BF),
            "bq": bq,
            "w_p": w_p.astype(_BF),
            "mask": mask.astype(_BF),
        })
    return in_maps


def _run(inputs, trace=False):
    from concourse.bass_utils import run_bass_kernel_spmd

    x = np.asarray(inputs["x"], dtype=np.float32)
    W_qkv = np.asarray(inputs["W_qkv"], dtype=np.float32)
    b_qkv = np.asarray(inputs["b_qkv"], dtype=np.float32)
    W_proj = np.asarray(inputs["W_proj"], dtype=np.float32)
    b_proj = np.asarray(inputs["b_proj"], dtype=np.float32)

    if "nc" not in _cache:
        _cache["nc"] = _build_nc()
    nc = _cache["nc"]

    in_maps = _prep_inputs(x, W_qkv, b_qkv, W_proj)
    res = run_bass_kernel_spmd(nc, in_maps, core_ids=list(range(NCORES)),
                               trace=trace)

    host_bias = b_proj + b_qkv[2 * D:3 * D] @ W_proj  # b_v folded through proj
    B = x.shape[0]
    out = np.zeros((B, S, D), dtype=np.float32)
    for cid in range(NCORES):
        b = cid // 4
        out[b] += res.results[cid]["out_p"]
    out += host_bias
    return out, res


def kernel(x, W_qkv, b_qkv, W_proj, b_proj):
    out, _ = _run({"x": x, "W_qkv": W_qkv, "b_qkv": b_qkv,
                   "W_proj": W_proj, "b_proj": b_proj})
    return out



# revision 4
# speedup vs baseline: 1.1380x; 1.1380x over previous
"""Multi-head self-attention (causal) Trainium2 Bass kernel, 8-core SPMD.

Sharding: 8 cores = 2 batches x 4 head-groups (3 heads each).
Each core computes, for its (batch, head-group):
  - Q^T, K^T, V projections from a host-pretransposed x^T (bf16)
  - causal attention in 256-query chunks: scores kept transposed (S^T[k,q]),
    all 3 heads' scores for one key-block exp'd in a single ACT instruction
    (PSUM [128,3,256] spanning 2 banks); softmax denominator comes free via
    a ones-column appended to V
  - AV flipped to out[q, d] orientation (cost = 65 rows/matmul instead of
    512), accumulated q-tile-major: each (q-tile, head) group runs its key
    blocks back-to-back into a ping-pong PSUM bank, then one fused divide
    (0-stride broadcast of the denominator column) normalizes into bf16
  - attn-out tiles are repartitioned back to [d, q] for the projection by
    DMA-engine transposes (16x128 xbar tiles, off the compute engines);
    the final chunk uses PE transposes to avoid DMA latency in the tail
  - its 3 heads' slice of the output projection (partial sum over d), bf16
Host gathers: out[b] = sum of 4 group partials + (b_proj + b_v @ W_proj).
b_k is dropped (softmax row-shift invariance), b_v folded into host bias.
"""

import numpy as np
import ml_dtypes

S = 2048          # sequence length
D = 768           # model dim
HD = 64           # head dim
HPC = 3           # heads per core
NCORES = 8
P = 128           # partitions
CT = D // P       # 6 contraction tiles over model dim
KT = S // P       # 16 key tiles
QC = 256          # query chunk
NQC = S // QC     # 8 query chunks

_BF = ml_dtypes.bfloat16

_cache = {}


def _build_nc():
    import concourse.bass as bass
    import concourse.mybir as mybir
    import concourse.tile as tile
    from concourse import bacc
    from contextlib import ExitStack

    bf = mybir.dt.bfloat16
    f32 = mybir.dt.float32

    nc = bacc.Bacc()
    xT = nc.declare_dram_parameter("xT", [D, S], bf, isOutput=False)
    # 3 lhsT slots per c-tile: 0=[Wk0|Wk1] 1=[Wq0|Wq1] 2=[Wk2|Wq2]
    w_qk = nc.declare_dram_parameter("w_qk", [D, 3, P], bf, isOutput=False)
    w_v = nc.declare_dram_parameter("w_v", [D, HPC * HD], bf, isOutput=False)
    # col 0: [bq_h0 | bq_h1]; col 1: rows 64:128 = bq_h2
    bq = nc.declare_dram_parameter("bq", [P, 2], f32, isOutput=False)
    # rows 0:128: W_proj rows of h0,h1; rows 128:192: h2
    w_p = nc.declare_dram_parameter("w_p", [HPC * HD, D], bf, isOutput=False)
    mask = nc.declare_dram_parameter("mask", [P, P], bf, isOutput=False)
    ident = nc.declare_dram_parameter("ident", [P, P], bf, isOutput=False)
    out_p = nc.declare_dram_parameter("out_p", [S, D], bf, isOutput=True)

    Exp = mybir.ActivationFunctionType.Exp

    def bcast_cols(ap, n):
        """Broadcast a [..., 1] AP along a new free dim of size n (0-stride)."""
        return bass.AP(tensor=ap.tensor, offset=ap.offset,
                       ap=[list(d) for d in ap.ap[:-1]] + [[0, n]])

    with tile.TileContext(nc) as tc, ExitStack() as ctx:
        singles = ctx.enter_context(tc.tile_pool(name="singles", bufs=1))
        # scores: [128,3,256] f32 = 2-bank slots
        ss_pool = ctx.enter_context(tc.tile_pool(name="ss", bufs=2, space="PSUM"))
        # AV accumulators: [128,65] f32, 1 bank each, ping-pong
        av_pool = ctx.enter_context(tc.tile_pool(name="av", bufs=2, space="PSUM"))
        # shared filler psum: qkv [128,256], proj [128,512]/[128,256], transposes
        fl_pool = ctx.enter_context(tc.tile_pool(name="fl", bufs=2, space="PSUM"))
        work = ctx.enter_context(tc.tile_pool(name="work", bufs=3))
        outs_pool = ctx.enter_context(tc.tile_pool(name="outs", bufs=3))

        # ---- persistent SBUF ----
        xT_s = singles.tile([P, CT, S], bf)
        wqk_s = singles.tile([P, CT, 3, P], bf)
        wv_s = singles.tile([P, CT, HPC * HD], bf)
        bq_s = singles.tile([P, 2], f32)
        mask_s = singles.tile([P, P], bf)
        ident_s = singles.tile([P, P], bf)
        wpa_s = singles.tile([P, D], bf)
        wpb_s = singles.tile([HD, D], bf)

        # preload the exp table while input DMAs are in flight
        dummy = singles.tile([P, 1], f32)
        nc.gpsimd.memset(dummy, 0.0)
        dummy2 = singles.tile([P, 1], f32)
        nc.scalar.activation(out=dummy2, in_=dummy, func=Exp, scale=1.0)

        # DMA order tuned so the first chunk's inputs land first.
        wqk_r = w_qk.rearrange("(t p) s m -> p t s m", p=P)
        nc.sync.dma_start(out=wqk_s, in_=wqk_r)
        xt_r = xT.rearrange("(t p) q -> p t q", p=P)
        nc.scalar.dma_start(out=xT_s[:, :, 0:QC], in_=xt_r[:, :, 0:QC])
        nc.gpsimd.dma_start(out=bq_s, in_=bq[:])
        nc.gpsimd.dma_start(out=mask_s, in_=mask[:])
        nc.gpsimd.dma_start(out=ident_s, in_=ident[:])
        wv_r = w_v.rearrange("(t p) m -> p t m", p=P)
        nc.gpsimd.dma_start(out=wv_s, in_=wv_r)
        nc.sync.dma_start(out=xT_s[:, :, QC:4 * QC], in_=xt_r[:, :, QC:4 * QC])
        nc.gpsimd.dma_start(out=xT_s[:, :, 4 * QC:S], in_=xt_r[:, :, 4 * QC:S])
        nc.gpsimd.dma_start(out=wpa_s, in_=w_p[0:P, :])
        nc.gpsimd.dma_start(out=wpb_s, in_=w_p[P:P + HD, :])

        # Q^T/K^T: slot 0 holds head0 (parts 0:64) + head1 (parts 64:128),
        # slot 1 holds head2 (parts 0:64; qt slot1 filled via repartition DMA).
        qt_s = singles.tile([P, 2, S], bf)
        kt_s = singles.tile([P, 2, S], bf)
        # V with a ones column appended per head (softmax denominator trick)
        v_s = singles.tile([P, KT, HPC, HD + 1], bf)
        nc.gpsimd.memset(v_s[:, :, :, HD:HD + 1], 1.0)
        # exp'd scores for a whole chunk, ping-pong by chunk parity:
        # [keys-part, key-tile, head, q-col]
        pt_s = [singles.tile([P, KT, HPC, QC], bf, name=f"pt{i}") for i in range(2)]
        # attn-out^T buffers: [h0 d|h1 d, q] and [h2 d|zeros, q]
        attn01T = singles.tile([P, S], bf)
        attn2T = singles.tile([P, S], bf)
        # attn-out staging [q, 4 slots, (h0|h1|h2|zeros) 256], transposed out
        ao_s = singles.tile([P, 4, 2 * P], bf)
        nc.gpsimd.memset(ao_s[:, :, HPC * HD:2 * P], 0.0)

        hsl = [slice(0, HD), slice(HD, P), slice(0, HD)]
        hslot = [0, 0, 1]

        def qkv_fillers(c):
            # Q^T/K^T/V projections for chunk c, as 7 PE filler groups
            qs = c * QC
            qsl = slice(qs, qs + QC)

            def g_kk():
                ps_kk = fl_pool.tile([P, QC], f32, tag="fl", name="ps_kk")
                for ct in range(CT):
                    nc.tensor.matmul(ps_kk, lhsT=wqk_s[:, ct, 0, :],
                                     rhs=xT_s[:, ct, qsl],
                                     start=(ct == 0), stop=(ct == CT - 1))
                nc.vector.tensor_copy(out=kt_s[:, 0, qsl], in_=ps_kk)

            def g_qq():
                ps_qq = fl_pool.tile([P, QC], f32, tag="fl", name="ps_qq")
                for ct in range(CT):
                    nc.tensor.matmul(ps_qq, lhsT=wqk_s[:, ct, 1, :],
                                     rhs=xT_s[:, ct, qsl],
                                     start=(ct == 0), stop=(ct == CT - 1))
                nc.vector.tensor_scalar_add(out=qt_s[:, 0, qsl], in0=ps_qq,
                                            scalar1=bq_s[:, 0:1])

            def g_kq2():
                ps_kq2 = fl_pool.tile([P, QC], f32, tag="fl", name="ps_kq2")
                for ct in range(CT):
                    nc.tensor.matmul(ps_kq2, lhsT=wqk_s[:, ct, 2, :],
                                     rhs=xT_s[:, ct, qsl],
                                     start=(ct == 0), stop=(ct == CT - 1))
                nc.vector.tensor_copy(out=kt_s[0:HD, 1, qsl], in_=ps_kq2[0:HD, :])
                # head2 Q lands in parts 64:128; bias-add, then repartition DMA
                q2st = work.tile([P, QC], bf, tag="q2st", name="q2st")
                nc.vector.tensor_scalar_add(out=q2st[HD:P, :], in0=ps_kq2[HD:P, :],
                                            scalar1=bq_s[HD:P, 1:2])
                nc.sync.dma_start(out=qt_s[0:HD, 1, qsl], in_=q2st[HD:P, :])

            def g_v(kt):
                def f():
                    ps_v = fl_pool.tile([P, HPC * HD], f32, tag="fl", name="ps_v")
                    for ct in range(CT):
                        nc.tensor.matmul(ps_v,
                                         lhsT=xT_s[:, ct, kt * P:(kt + 1) * P],
                                         rhs=wv_s[:, ct, :],
                                         start=(ct == 0), stop=(ct == CT - 1))
                    nc.vector.tensor_copy(
                        out=v_s[:, kt, :, 0:HD],
                        in_=ps_v.rearrange("p (h d) -> p h d", h=HPC))
                return f
            return [g_kk, g_qq, g_kq2] + [g_v(kt) for kt in (2 * c, 2 * c + 1)]

        def proj_fillers(t, last=False):
            # output projection of q-tile t, as 2 column-group pieces that
            # share one bf16 staging tile; the second piece sends the DMA
            holder = {}

            def piece(e0, en):
                def f():
                    pp = fl_pool.tile([P, en], f32, tag="fl", name="pp")
                    nc.tensor.matmul(pp, lhsT=attn01T[:, t * P:(t + 1) * P],
                                     rhs=wpa_s[:, e0:e0 + en],
                                     start=True, stop=False)
                    nc.tensor.matmul(pp, lhsT=attn2T[0:HD, t * P:(t + 1) * P],
                                     rhs=wpb_s[:, e0:e0 + en],
                                     start=False, stop=True)
                    if e0 == 0:
                        holder["ob"] = outs_pool.tile([P, D], bf, tag="ob",
                                                      name="ob")
                    ob = holder["ob"]
                    nc.vector.tensor_copy(out=ob[:, e0:e0 + en], in_=pp)
                    if e0 != 0:
                        nc.sync.dma_start(out=out_p[t * P:(t + 1) * P, :], in_=ob)
                return f
            return [piece(0, 512), piece(512, 256)]

        def emit_scores(kt, c, ss_t):
            qs = c * QC
            off = P if kt == 2 * c + 1 else 0
            n = QC - off
            for h in range(HPC):
                nc.tensor.matmul(ss_t[:, h, 0:n],
                                 lhsT=kt_s[hsl[h], hslot[h], kt * P:(kt + 1) * P],
                                 rhs=qt_s[hsl[h], hslot[h], qs + off:qs + QC],
                                 start=True, stop=True)

        def emit_exp_mask(kt, c, ss_t, pt):
            off = P if kt == 2 * c + 1 else 0
            n = QC - off
            nc.scalar.activation(out=pt[:, kt, :, off:QC], in_=ss_t[:, :, 0:n],
                                 func=Exp, scale=0.125)
            if kt >= 2 * c:  # diagonal block: mask k>q inside the 128x128 square
                for h in range(HPC):
                    nc.gpsimd.tensor_mul(out=pt[:, kt, h, off:off + P],
                                         in0=pt[:, kt, h, off:off + P], in1=mask_s)

        def emit_av_tile(t, c, pt, pe_transpose=False):
            # AV for q-tile t (flipped: out [q, d+1]), one head at a time,
            # then fused normalize-divide into the ao staging slot, then
            # repartition via DMA xbar transpose (PE transpose in the tail).
            qi = t - 2 * c
            slot = t % 4
            for h in range(HPC):
                po = av_pool.tile([P, HD + 1], f32, tag="av", name="po")
                nkt = 2 * c + qi + 1
                for kt in range(nkt):
                    nc.tensor.matmul(po, lhsT=pt[:, kt, h, qi * P:(qi + 1) * P],
                                     rhs=v_s[:, kt, h, :],
                                     start=(kt == 0), stop=(kt == nkt - 1))
                nc.vector.tensor_scalar(out=ao_s[:, slot, h * HD:(h + 1) * HD],
                                        in0=po[:, 0:HD],
                                        scalar1=po[:, HD:HD + 1], scalar2=None,
                                        op0=mybir.AluOpType.divide)
            tsl = slice(t * P, (t + 1) * P)
            if pe_transpose:
                for half in range(2):
                    tp = fl_pool.tile([P, P], bf, tag="fl", name="tp")
                    nc.tensor.transpose(tp, ao_s[:, slot, half * P:(half + 1) * P],
                                        ident_s)
                    dst = attn01T if half == 0 else attn2T
                    nc.vector.tensor_copy(out=dst[:, tsl], in_=tp)
            else:
                nc.sync.dma_start_transpose(attn01T[:, tsl], ao_s[:, slot, 0:P])
                nc.sync.dma_start_transpose(attn2T[:, tsl], ao_s[:, slot, P:2 * P])

        for f in qkv_fillers(0):
            f()

        for c in range(NQC):
            nkt = 2 * c + 2
            pt = pt_s[c % 2]
            last = c == NQC - 1
            # fillers woven into this chunk's attention: next chunk's QKV,
            # then the previous chunk's projections
            fillers = []
            if not last:
                fillers += qkv_fillers(c + 1)
            if c > 0:
                for t in (2 * (c - 1), 2 * (c - 1) + 1):
                    fillers += proj_fillers(t)
            emitted = 0
            n_fill = len(fillers)

            prev = None
            for kt in range(nkt):
                ss_t = ss_pool.tile([P, HPC, QC], f32, tag="ss", name="ss_t")
                emit_scores(kt, c, ss_t)
                if prev is not None:
                    emit_exp_mask(kt - 1, c, prev, pt)
                prev = ss_t
                want = ((kt + 1) * n_fill) // (nkt + 1)
                while emitted < want:
                    fillers[emitted]()
                    emitted += 1
                if kt == nkt - 1:
                    # q-tile 2c only needs key blocks <= 2c: runs during the
                    # last key block's scores/exp
                    emit_exp_mask(kt, c, prev, pt)
                    prev = None
                    emit_av_tile(2 * c, c, pt, pe_transpose=last)
            while emitted < n_fill:
                fillers[emitted]()
                emitted += 1
            emit_av_tile(2 * c + 1, c, pt, pe_transpose=last)

        # tail: projections of the final two q-tiles
        for t in (2 * (NQC - 1), 2 * (NQC - 1) + 1):
            for f in proj_fillers(t, last=True):
                f()

    nc.compile()
    return nc


def _prep_inputs(x, W_qkv, b_qkv, W_proj):
    """Build the 8 per-core input maps (all bf16 except biases)."""
    in_maps = []
    for cid in range(NCORES):
        b, g = divmod(cid, 4)
        hs = [g * HPC + i for i in range(HPC)]  # global head ids

        def wslice(kind, h):  # kind 0=q 1=k 2=v
            return W_qkv[:, kind * D + h * HD:(kind * D + (h + 1) * HD)]

        xT = np.ascontiguousarray(x[b].T).astype(_BF)

        w_qk = np.zeros((D, 3, P), dtype=np.float32)
        w_qk[:, 0, 0:HD] = wslice(1, hs[0])
        w_qk[:, 0, HD:P] = wslice(1, hs[1])
        w_qk[:, 1, 0:HD] = wslice(0, hs[0])
        w_qk[:, 1, HD:P] = wslice(0, hs[1])
        w_qk[:, 2, 0:HD] = wslice(1, hs[2])
        w_qk[:, 2, HD:P] = wslice(0, hs[2])

        w_v = np.concatenate([wslice(2, h) for h in hs], axis=1)

        bq = np.zeros((P, 2), dtype=np.float32)
        bq[0:HD, 0] = b_qkv[hs[0] * HD:(hs[0] + 1) * HD]
        bq[HD:P, 0] = b_qkv[hs[1] * HD:(hs[1] + 1) * HD]
        bq[HD:P, 1] = b_qkv[hs[2] * HD:(hs[2] + 1) * HD]

        w_p = np.concatenate([W_proj[h * HD:(h + 1) * HD, :] for h in hs], axis=0)

        mask = np.triu(np.ones((P, P), dtype=np.float32))

        in_maps.append({
            "xT": xT,
            "w_qk": w_qk.astype(_BF),
            "w_v": w_v.astype(_BF),
            "bq": bq,
            "w_p": w_p.astype(_BF),
            "mask": mask.astype(_BF),
            "ident": np.eye(P, dtype=np.float32).astype(_BF),
        })
    return in_maps


def _run(inputs, trace=False):
    from concourse.bass_utils import run_bass_kernel_spmd

    x = np.asarray(inputs["x"], dtype=np.float32)
    W_qkv = np.asarray(inputs["W_qkv"], dtype=np.float32)
    b_qkv = np.asarray(inputs["b_qkv"], dtype=np.float32)
    W_proj = np.asarray(inputs["W_proj"], dtype=np.float32)
    b_proj = np.asarray(inputs["b_proj"], dtype=np.float32)

    if "nc" not in _cache:
        _cache["nc"] = _build_nc()
    nc = _cache["nc"]

    in_maps = _prep_inputs(x, W_qkv, b_qkv, W_proj)
    res = run_bass_kernel_spmd(nc, in_maps, core_ids=list(range(NCORES)),
                               trace=trace)

    host_bias = b_proj + b_qkv[2 * D:3 * D] @ W_proj  # b_v folded through proj
    B = x.shape[0]
    out = np.zeros((B, S, D), dtype=np.float32)
    for cid in range(NCORES):
        b = cid // 4
        out[b] += res.results[cid]["out_p"].astype(np.float32)
    out += host_bias
    return out, res


def kernel(x, W_qkv, b_qkv, W_proj, b_proj):
    out, _ = _run({"x": x, "W_qkv": W_qkv, "b_qkv": b_qkv,
                   "W_proj": W_proj, "b_proj": b_proj})
    return out


# revision 6
# speedup vs baseline: 1.1907x; 1.0464x over previous
"""Multi-head self-attention (causal) Trainium2 Bass kernel, 8-core SPMD.

Sharding: 8 cores = 2 batches x 4 head-groups (3 heads each).
Each core computes, for its (batch, head-group):
  - Q^T, K^T, V projections from a host-pretransposed x^T (bf16)
  - causal attention in 256-query chunks: scores kept transposed (S^T[k,q]),
    all 3 heads' scores for one key-block exp'd in a single ACT instruction
    (PSUM [128,3,256] spanning 2 banks); softmax denominator comes free via
    a ones-column appended to V
  - AV flipped to out[q, d] orientation (cost = 65 rows/matmul instead of
    512), accumulated q-tile-major: each (q-tile, head) group runs its key
    blocks back-to-back into a ping-pong PSUM bank, then one fused divide
    (0-stride broadcast of the denominator column) normalizes into bf16
  - attn-out tiles are repartitioned back to [d, q] for the projection by
    DMA-engine transposes (16x128 xbar tiles, off the compute engines);
    the final chunk uses PE transposes to avoid DMA latency in the tail
  - its 3 heads' slice of the output projection (partial sum over d), bf16
Host gathers: out[b] = sum of 4 group partials + (b_proj + b_v @ W_proj).
b_k is dropped (softmax row-shift invariance), b_v folded into host bias.
"""

import numpy as np
import ml_dtypes

S = 2048          # sequence length
D = 768           # model dim
HD = 64           # head dim
HPC = 3           # heads per core
NCORES = 8
P = 128           # partitions
CT = D // P       # 6 contraction tiles over model dim
KT = S // P       # 16 key tiles
QC = 256          # query chunk
NQC = S // QC     # 8 query chunks

_BF = ml_dtypes.bfloat16

_cache = {}


def _build_nc():
    import concourse.bass as bass
    import concourse.mybir as mybir
    import concourse.tile as tile
    from concourse import bacc
    from contextlib import ExitStack

    bf = mybir.dt.bfloat16
    f32 = mybir.dt.float32

    nc = bacc.Bacc()
    xT = nc.declare_dram_parameter("xT", [D, S], bf, isOutput=False)
    # 3 lhsT slots per c-tile: 0=[Wk0|Wk1] 1=[Wq0|Wq1] 2=[Wk2|Wq2]
    w_qk = nc.declare_dram_parameter("w_qk", [D, 3, P], bf, isOutput=False)
    w_v = nc.declare_dram_parameter("w_v", [D, HPC * HD], bf, isOutput=False)
    # col 0: [bq_h0 | bq_h1]; col 1: rows 64:128 = bq_h2
    bq = nc.declare_dram_parameter("bq", [P, 2], f32, isOutput=False)
    # rows 0:128: W_proj rows of h0,h1; rows 128:192: h2
    w_p = nc.declare_dram_parameter("w_p", [HPC * HD, D], bf, isOutput=False)
    mask = nc.declare_dram_parameter("mask", [P, P], bf, isOutput=False)
    ident = nc.declare_dram_parameter("ident", [P, P], bf, isOutput=False)
    out_p = nc.declare_dram_parameter("out_p", [S, D], bf, isOutput=True)

    Exp = mybir.ActivationFunctionType.Exp

    def bcast_cols(ap, n):
        """Broadcast a [..., 1] AP along a new free dim of size n (0-stride)."""
        return bass.AP(tensor=ap.tensor, offset=ap.offset,
                       ap=[list(d) for d in ap.ap[:-1]] + [[0, n]])

    with tile.TileContext(nc) as tc, ExitStack() as ctx:
        singles = ctx.enter_context(tc.tile_pool(name="singles", bufs=1))
        # scores: [128,3,256] f32 = 2-bank slots
        ss_pool = ctx.enter_context(tc.tile_pool(name="ss", bufs=2, space="PSUM"))
        # AV accumulators: [128,65] f32, 1 bank each, ping-pong
        av_pool = ctx.enter_context(tc.tile_pool(name="av", bufs=2, space="PSUM"))
        # shared filler psum: qkv [128,256], proj [128,512]/[128,256], transposes
        fl_pool = ctx.enter_context(tc.tile_pool(name="fl", bufs=2, space="PSUM"))
        work = ctx.enter_context(tc.tile_pool(name="work", bufs=3))
        outs_pool = ctx.enter_context(tc.tile_pool(name="outs", bufs=3))

        # ---- persistent SBUF ----
        xT_s = singles.tile([P, CT, S], bf)
        wqk_s = singles.tile([P, CT, 3, P], bf)
        wv_s = singles.tile([P, CT, HPC * HD], bf)
        bq_s = singles.tile([P, 2], f32)
        mask_s = singles.tile([P, P], bf)
        ident_s = singles.tile([P, P], bf)
        wpa_s = singles.tile([P, D], bf)
        wpb_s = singles.tile([HD, D], bf)

        # preload the exp table while input DMAs are in flight
        dummy = singles.tile([P, 1], f32)
        nc.gpsimd.memset(dummy, 0.0)
        dummy2 = singles.tile([P, 1], f32)
        nc.scalar.activation(out=dummy2, in_=dummy, func=Exp, scale=1.0)

        # DMA order tuned so the first chunk's inputs land first.
        wqk_r = w_qk.rearrange("(t p) s m -> p t s m", p=P)
        nc.sync.dma_start(out=wqk_s, in_=wqk_r)
        xt_r = xT.rearrange("(t p) q -> p t q", p=P)
        nc.scalar.dma_start(out=xT_s[:, :, 0:QC], in_=xt_r[:, :, 0:QC])
        nc.gpsimd.dma_start(out=bq_s, in_=bq[:])
        nc.gpsimd.dma_start(out=mask_s, in_=mask[:])
        nc.gpsimd.dma_start(out=ident_s, in_=ident[:])
        wv_r = w_v.rearrange("(t p) m -> p t m", p=P)
        nc.gpsimd.dma_start(out=wv_s, in_=wv_r)
        nc.sync.dma_start(out=xT_s[:, :, QC:4 * QC], in_=xt_r[:, :, QC:4 * QC])
        nc.gpsimd.dma_start(out=xT_s[:, :, 4 * QC:S], in_=xt_r[:, :, 4 * QC:S])
        nc.gpsimd.dma_start(out=wpa_s, in_=w_p[0:P, :])
        nc.gpsimd.dma_start(out=wpb_s, in_=w_p[P:P + HD, :])

        # Q^T/K^T: slot 0 holds head0 (parts 0:64) + head1 (parts 64:128),
        # slot 1 holds head2 (parts 0:64; qt slot1 filled via repartition DMA).
        qt_s = singles.tile([P, 2, S], bf)
        kt_s = singles.tile([P, 2, S], bf)
        # V with a ones column appended per head (softmax denominator trick)
        v_s = singles.tile([P, KT, HPC, HD + 1], bf)
        nc.gpsimd.memset(v_s[:, :, :, HD:HD + 1], 1.0)
        # exp'd scores for a whole chunk, ping-pong by chunk parity:
        # [keys-part, key-tile, head, q-col]
        pt_s = [singles.tile([P, KT, HPC, QC], bf, name=f"pt{i}") for i in range(2)]
        # attn-out^T buffers: [h0 d|h1 d, q] and [h2 d|zeros, q]
        attn01T = singles.tile([P, S], bf)
        attn2T = singles.tile([P, S], bf)
        # attn-out staging [q, 4 slots, (h0|h1|h2|zeros) 256], transposed out
        ao_s = singles.tile([P, 4, 2 * P], bf)
        nc.gpsimd.memset(ao_s[:, :, HPC * HD:2 * P], 0.0)

        hsl = [slice(0, HD), slice(HD, P), slice(0, HD)]
        hslot = [0, 0, 1]

        def qkv_fillers(c):
            # Q^T/K^T/V projections for chunk c, as 7 PE filler groups
            qs = c * QC
            qsl = slice(qs, qs + QC)

            def g_kk():
                ps_kk = fl_pool.tile([P, QC], f32, tag="fl", name="ps_kk")
                for ct in range(CT):
                    nc.tensor.matmul(ps_kk, lhsT=wqk_s[:, ct, 0, :],
                                     rhs=xT_s[:, ct, qsl],
                                     start=(ct == 0), stop=(ct == CT - 1))
                nc.vector.tensor_copy(out=kt_s[:, 0, qsl], in_=ps_kk)

            def g_qq():
                ps_qq = fl_pool.tile([P, QC], f32, tag="fl", name="ps_qq")
                for ct in range(CT):
                    nc.tensor.matmul(ps_qq, lhsT=wqk_s[:, ct, 1, :],
                                     rhs=xT_s[:, ct, qsl],
                                     start=(ct == 0), stop=(ct == CT - 1))
                nc.vector.tensor_scalar_add(out=qt_s[:, 0, qsl], in0=ps_qq,
                                            scalar1=bq_s[:, 0:1])

            def g_kq2():
                ps_kq2 = fl_pool.tile([P, QC], f32, tag="fl", name="ps_kq2")
                for ct in range(CT):
                    nc.tensor.matmul(ps_kq2, lhsT=wqk_s[:, ct, 2, :],
                                     rhs=xT_s[:, ct, qsl],
                                     start=(ct == 0), stop=(ct == CT - 1))
                nc.vector.tensor_copy(out=kt_s[0:HD, 1, qsl], in_=ps_kq2[0:HD, :])
                # head2 Q lands in parts 64:128; bias-add, then repartition DMA
                q2st = work.tile([P, QC], bf, tag="q2st", name="q2st")
                nc.vector.tensor_scalar_add(out=q2st[HD:P, :], in0=ps_kq2[HD:P, :],
                                            scalar1=bq_s[HD:P, 1:2])
                nc.sync.dma_start(out=qt_s[0:HD, 1, qsl], in_=q2st[HD:P, :])

            def g_v(kt):
                def f():
                    ps_v = fl_pool.tile([P, HPC * HD], f32, tag="fl", name="ps_v")
                    for ct in range(CT):
                        nc.tensor.matmul(ps_v,
                                         lhsT=xT_s[:, ct, kt * P:(kt + 1) * P],
                                         rhs=wv_s[:, ct, :],
                                         start=(ct == 0), stop=(ct == CT - 1))
                    nc.vector.tensor_copy(
                        out=v_s[:, kt, :, 0:HD],
                        in_=ps_v.rearrange("p (h d) -> p h d", h=HPC))
                return f
            return [g_kk, g_qq, g_kq2] + [g_v(kt) for kt in (2 * c, 2 * c + 1)]

        def proj_fillers(t, last=False):
            # output projection of q-tile t, as 2 column-group pieces that
            # share one bf16 staging tile; the second piece sends the DMA
            holder = {}

            def piece(e0, en):
                def f():
                    pp = fl_pool.tile([P, en], f32, tag="fl", name="pp")
                    nc.tensor.matmul(pp, lhsT=attn01T[:, t * P:(t + 1) * P],
                                     rhs=wpa_s[:, e0:e0 + en],
                                     start=True, stop=False)
                    nc.tensor.matmul(pp, lhsT=attn2T[0:HD, t * P:(t + 1) * P],
                                     rhs=wpb_s[:, e0:e0 + en],
                                     start=False, stop=True)
                    if e0 == 0:
                        holder["ob"] = outs_pool.tile([P, D], bf, tag="ob",
                                                      name="ob")
                    ob = holder["ob"]
                    nc.vector.tensor_copy(out=ob[:, e0:e0 + en], in_=pp)
                    if e0 != 0:
                        nc.sync.dma_start(out=out_p[t * P:(t + 1) * P, :], in_=ob)
                return f
            return [piece(0, 512), piece(512, 256)]

        def emit_scores(kt, c, ss_t):
            qs = c * QC
            off = P if kt == 2 * c + 1 else 0
            n = QC - off
            for h in range(HPC):
                nc.tensor.matmul(ss_t[:, h, 0:n],
                                 lhsT=kt_s[hsl[h], hslot[h], kt * P:(kt + 1) * P],
                                 rhs=qt_s[hsl[h], hslot[h], qs + off:qs + QC],
                                 start=True, stop=True)

        def emit_exp_mask(kt, c, ss_t, pt):
            off = P if kt == 2 * c + 1 else 0
            n = QC - off
            nc.scalar.activation(out=pt[:, kt, :, off:QC], in_=ss_t[:, :, 0:n],
                                 func=Exp, scale=0.125)
            if kt >= 2 * c:  # diagonal block: mask k>q inside the 128x128 square
                for h in range(HPC):
                    nc.gpsimd.tensor_mul(out=pt[:, kt, h, off:off + P],
                                         in0=pt[:, kt, h, off:off + P], in1=mask_s)

        def emit_av_tile(t, c, pt, pe_transpose=False):
            # AV for q-tile t (flipped: out [q, d+1]), one head at a time,
            # then fused normalize-divide into the ao staging slot, then
            # repartition via DMA xbar transpose (PE transpose in the tail).
            qi = t - 2 * c
            slot = t % 4
            for h in range(HPC):
                po = av_pool.tile([P, HD + 1], f32, tag="av", name="po")
                nkt = 2 * c + qi + 1
                for kt in range(nkt):
                    nc.tensor.matmul(po, lhsT=pt[:, kt, h, qi * P:(qi + 1) * P],
                                     rhs=v_s[:, kt, h, :],
                                     start=(kt == 0), stop=(kt == nkt - 1))
                nc.vector.tensor_scalar(out=ao_s[:, slot, h * HD:(h + 1) * HD],
                                        in0=po[:, 0:HD],
                                        scalar1=po[:, HD:HD + 1], scalar2=None,
                                        op0=mybir.AluOpType.divide)
            tsl = slice(t * P, (t + 1) * P)
            if pe_transpose:
                for half in range(2):
                    tp = fl_pool.tile([P, P], bf, tag="fl", name="tp")
                    nc.tensor.transpose(tp, ao_s[:, slot, half * P:(half + 1) * P],
                                        ident_s)
                    dst = attn01T if half == 0 else attn2T
                    nc.vector.tensor_copy(out=dst[:, tsl], in_=tp)
            else:
                nc.sync.dma_start_transpose(attn01T[:, tsl], ao_s[:, slot, 0:P])
                nc.sync.dma_start_transpose(attn2T[:, tsl], ao_s[:, slot, P:2 * P])

        for f in qkv_fillers(0):
            f()

        # Projection tiles are deferred toward the late, exp-heavy chunks
        # where the PE would otherwise starve waiting on ACT. proj(t) may run
        # any chunk after t's transposes (end of chunk t//2); quotas sized to
        # each chunk's PE-vs-ACT deficit.
        proj_quota = {4: 1, 5: 2, 6: 3, 7: 8}
        proj_next = 0  # next q-tile whose projection is still unscheduled

        for c in range(NQC):
            nkt = 2 * c + 2
            pt = pt_s[c % 2]
            last = c == NQC - 1
            # fillers woven into this chunk's attention: next chunk's QKV,
            # then deferred projections (ready through q-tile 2c-1)
            fillers = []
            if not last:
                fillers += qkv_fillers(c + 1)
            ready = 2 * c  # q-tiles with transposes complete before chunk c
            for _ in range(proj_quota.get(c, 0)):
                if proj_next < ready:
                    fillers += proj_fillers(proj_next)
                    proj_next += 1
            emitted = 0
            n_fill = len(fillers)

            prev = None
            for kt in range(nkt):
                ss_t = ss_pool.tile([P, HPC, QC], f32, tag="ss", name="ss_t")
                emit_scores(kt, c, ss_t)
                if prev is not None:
                    emit_exp_mask(kt - 1, c, prev, pt)
                prev = ss_t
                want = ((kt + 1) * n_fill) // (nkt + 1)
                while emitted < want:
                    fillers[emitted]()
                    emitted += 1
                if kt == nkt - 1:
                    # q-tile 2c only needs key blocks <= 2c: runs during the
                    # last key block's scores/exp
                    emit_exp_mask(kt, c, prev, pt)
                    prev = None
                    emit_av_tile(2 * c, c, pt, pe_transpose=last)
            while emitted < n_fill:
                fillers[emitted]()
                emitted += 1
            emit_av_tile(2 * c + 1, c, pt, pe_transpose=last)

        # tail: any remaining projections (final two q-tiles at least)
        for t in range(proj_next, 2 * NQC):
            for f in proj_fillers(t, last=True):
                f()

    nc.compile()
    return nc


def _prep_inputs(x, W_qkv, b_qkv, W_proj):
    """Build the 8 per-core input maps (all bf16 except biases)."""
    in_maps = []
    for cid in range(NCORES):
        b, g = divmod(cid, 4)
        hs = [g * HPC + i for i in range(HPC)]  # global head ids

        def wslice(kind, h):  # kind 0=q 1=k 2=v
            return W_qkv[:, kind * D + h * HD:(kind * D + (h + 1) * HD)]

        xT = np.ascontiguousarray(x[b].T).astype(_BF)

        w_qk = np.zeros((D, 3, P), dtype=np.float32)
        w_qk[:, 0, 0:HD] = wslice(1, hs[0])
        w_qk[:, 0, HD:P] = wslice(1, hs[1])
        w_qk[:, 1, 0:HD] = wslice(0, hs[0])
        w_qk[:, 1, HD:P] = wslice(0, hs[1])
        w_qk[:, 2, 0:HD] = wslice(1, hs[2])
        w_qk[:, 2, HD:P] = wslice(0, hs[2])

        w_v = np.concatenate([wslice(2, h) for h in hs], axis=1)

        bq = np.zeros((P, 2), dtype=np.float32)
        bq[0:HD, 0] = b_qkv[hs[0] * HD:(hs[0] + 1) * HD]
        bq[HD:P, 0] = b_qkv[hs[1] * HD:(hs[1] + 1) * HD]
        bq[HD:P, 1] = b_qkv[hs[2] * HD:(hs[2] + 1) * HD]

        w_p = np.concatenate([W_proj[h * HD:(h + 1) * HD, :] for h in hs], axis=0)

        mask = np.triu(np.ones((P, P), dtype=np.float32))

        in_maps.append({
            "xT": xT,
            "w_qk": w_qk.astype(_BF),
            "w_v": w_v.astype(_BF),
            "bq": bq,
            "w_p": w_p.astype(_BF),
            "mask": mask.astype(_BF),
            "ident": np.eye(P, dtype=np.float32).astype(_BF),
        })
    return in_maps


def _run(inputs, trace=False):
    from concourse.bass_utils import run_bass_kernel_spmd

    x = np.asarray(inputs["x"], dtype=np.float32)
    W_qkv = np.asarray(inputs["W_qkv"], dtype=np.float32)
    b_qkv = np.asarray(inputs["b_qkv"], dtype=np.float32)
    W_proj = np.asarray(inputs["W_proj"], dtype=np.float32)
    b_proj = np.asarray(inputs["b_proj"], dtype=np.float32)

    if "nc" not in _cache:
        _cache["nc"] = _build_nc()
    nc = _cache["nc"]

    in_maps = _prep_inputs(x, W_qkv, b_qkv, W_proj)
    res = run_bass_kernel_spmd(nc, in_maps, core_ids=list(range(NCORES)),
                               trace=trace)

    host_bias = b_proj + b_qkv[2 * D:3 * D] @ W_proj  # b_v folded through proj
    B = x.shape[0]
    out = np.zeros((B, S, D), dtype=np.float32)
    for cid in range(NCORES):
        b = cid // 4
        out[b] += res.results[cid]["out_p"].astype(np.float32)
    out += host_bias
    return out, res


def kernel(x, W_qkv, b_qkv, W_proj, b_proj):
    out, _ = _run({"x": x, "W_qkv": W_qkv, "b_qkv": b_qkv,
                   "W_proj": W_proj, "b_proj": b_proj})
    return out


# revision 17
# speedup vs baseline: 1.2123x; 1.0181x over previous
"""Multi-head self-attention (causal) Trainium2 Bass kernel, 8-core SPMD.

Sharding: 8 cores = 2 batches x 4 head-groups (3 heads each).
Each core computes, for its (batch, head-group):
  - Q^T, K^T, V projections from a host-pretransposed x^T (bf16)
  - causal attention in 256-query chunks: scores kept transposed (S^T[k,q]),
    all 3 heads' scores for one key-block exp'd in a single ACT instruction
    (PSUM [128,3,256] spanning 2 banks); softmax denominator comes free via
    a ones-column appended to V
  - AV flipped to out[q, d] orientation (cost = 65 rows/matmul instead of
    512), accumulated q-tile-major: each (q-tile, head) group runs its key
    blocks back-to-back into a ping-pong PSUM bank, then one fused divide
    (0-stride broadcast of the denominator column) normalizes into bf16
  - attn-out tiles are repartitioned back to [d, q] for the projection by
    DMA-engine transposes (16x128 xbar tiles, off the compute engines);
    the final chunk uses PE transposes to avoid DMA latency in the tail
  - its 3 heads' slice of the output projection (partial sum over d), bf16
Host gathers: out[b] = sum of 4 group partials + (b_proj + b_v @ W_proj).
b_k is dropped (softmax row-shift invariance), b_v folded into host bias.
"""

import numpy as np
import ml_dtypes

S = 2048          # sequence length
D = 768           # model dim
HD = 64           # head dim
HPC = 3           # heads per core
NCORES = 8
P = 128           # partitions
CT = D // P       # 6 contraction tiles over model dim
KT = S // P       # 16 key tiles
QC = 256          # query chunk
NQC = S // QC     # 8 query chunks

_BF = ml_dtypes.bfloat16

_cache = {}


def _build_nc():
    import concourse.bass as bass
    import concourse.mybir as mybir
    import concourse.tile as tile
    from concourse import bacc
    from contextlib import ExitStack

    bf = mybir.dt.bfloat16
    f32 = mybir.dt.float32

    nc = bacc.Bacc()
    xT = nc.declare_dram_parameter("xT", [D, S], bf, isOutput=False)
    # 3 lhsT slots per c-tile: 0=[Wk0|Wk1] 1=[Wq0|Wq1] 2=[Wk2|Wq2]
    w_qk = nc.declare_dram_parameter("w_qk", [D, 3, P], bf, isOutput=False)
    w_v = nc.declare_dram_parameter("w_v", [D, HPC * HD], bf, isOutput=False)
    # col 0: [bq_h0 | bq_h1]; col 1: rows 64:128 = bq_h2
    bq = nc.declare_dram_parameter("bq", [P, 2], f32, isOutput=False)
    # rows 0:128: W_proj rows of h0,h1; rows 128:192: h2
    w_p = nc.declare_dram_parameter("w_p", [HPC * HD, D], bf, isOutput=False)
    mask = nc.declare_dram_parameter("mask", [P, P], bf, isOutput=False)
    ident = nc.declare_dram_parameter("ident", [P, P], bf, isOutput=False)
    out_p = nc.declare_dram_parameter("out_p", [S, D], bf, isOutput=True)

    Exp = mybir.ActivationFunctionType.Exp

    def bcast_cols(ap, n):
        """Broadcast a [..., 1] AP along a new free dim of size n (0-stride)."""
        return bass.AP(tensor=ap.tensor, offset=ap.offset,
                       ap=[list(d) for d in ap.ap[:-1]] + [[0, n]])

    with tile.TileContext(nc) as tc, ExitStack() as ctx:
        singles = ctx.enter_context(tc.tile_pool(name="singles", bufs=1))
        # scores: [128,3,256] f32 = 2-bank slots
        ss_pool = ctx.enter_context(tc.tile_pool(name="ss", bufs=2, space="PSUM"))
        # AV accumulators: [128,65] f32, 1 bank each, ping-pong
        av_pool = ctx.enter_context(tc.tile_pool(name="av", bufs=2, space="PSUM"))
        # shared filler psum: qkv [128,256], proj [128,512]/[128,256], transposes
        fl_pool = ctx.enter_context(tc.tile_pool(name="fl", bufs=2, space="PSUM"))
        work = ctx.enter_context(tc.tile_pool(name="work", bufs=3))
        outs_pool = ctx.enter_context(tc.tile_pool(name="outs", bufs=3))

        # ---- persistent SBUF ----
        xT_s = singles.tile([P, CT, S], bf)
        wqk_s = singles.tile([P, CT, 3, P], bf)
        wv_s = singles.tile([P, CT, HPC * HD], bf)
        bq_s = singles.tile([P, 2], f32)
        mask_s = singles.tile([P, P], bf)
        ident_s = singles.tile([P, P], bf)
        wpa_s = singles.tile([P, D], bf)
        wpb_s = singles.tile([HD, D], bf)

        # preload the exp table while input DMAs are in flight
        dummy = singles.tile([P, 1], f32)
        nc.gpsimd.memset(dummy, 0.0)
        dummy2 = singles.tile([P, 1], f32)
        nc.scalar.activation(out=dummy2, in_=dummy, func=Exp, scale=1.0)

        # DMA order tuned so the first chunk's inputs land first: the kk
        # group needs the K weight slot + x^T's first 256 columns, split
        # across both HWDGE queues to overlap issue latency.
        wqk_r = w_qk.rearrange("(t p) s m -> p t s m", p=P)
        xt_r = xT.rearrange("(t p) q -> p t q", p=P)
        nc.sync.dma_start(out=wqk_s[:, :, 0:1, :], in_=wqk_r[:, :, 0:1, :])
        nc.scalar.dma_start(out=xT_s[:, 0:3, 0:QC], in_=xt_r[:, 0:3, 0:QC])
        nc.sync.dma_start(out=xT_s[:, 3:CT, 0:QC], in_=xt_r[:, 3:CT, 0:QC])
        nc.scalar.dma_start(out=wqk_s[:, :, 1:3, :], in_=wqk_r[:, :, 1:3, :])
        nc.gpsimd.dma_start(out=bq_s, in_=bq[:])
        nc.gpsimd.dma_start(out=mask_s, in_=mask[:])
        nc.gpsimd.dma_start(out=ident_s, in_=ident[:])
        wv_r = w_v.rearrange("(t p) m -> p t m", p=P)
        nc.gpsimd.dma_start(out=wv_s, in_=wv_r)
        nc.sync.dma_start(out=xT_s[:, :, QC:4 * QC], in_=xt_r[:, :, QC:4 * QC])
        nc.gpsimd.dma_start(out=xT_s[:, :, 4 * QC:S], in_=xt_r[:, :, 4 * QC:S])
        nc.gpsimd.dma_start(out=wpa_s, in_=w_p[0:P, :])
        nc.gpsimd.dma_start(out=wpb_s, in_=w_p[P:P + HD, :])

        # Q^T/K^T: slot 0 holds head0 (parts 0:64) + head1 (parts 64:128),
        # slot 1 holds head2 (parts 0:64; qt slot1 filled via repartition DMA).
        qt_s = singles.tile([P, 2, S], bf)
        kt_s = singles.tile([P, 2, S], bf)
        # V with a ones column appended per head (softmax denominator trick)
        v_s = singles.tile([P, KT, HPC, HD + 1], bf)
        nc.gpsimd.memset(v_s[:, :, :, HD:HD + 1], 1.0)
        # exp'd scores for a whole chunk, ping-pong by chunk parity:
        # [keys-part, key-tile, head, q-col]
        pt_s = [singles.tile([P, KT, HPC, QC], bf, name=f"pt{i}") for i in range(2)]
        # attn-out^T buffers: [h0 d|h1 d, q] and [h2 d|zeros, q]
        attn01T = singles.tile([P, S], bf)
        attn2T = singles.tile([P, S], bf)
        # attn-out staging [q, 4 slots, (h0|h1|h2|zeros) 256], transposed out
        ao_s = singles.tile([P, 4, 2 * P], bf)
        nc.gpsimd.memset(ao_s[:, :, HPC * HD:2 * P], 0.0)

        hsl = [slice(0, HD), slice(HD, P), slice(0, HD)]
        hslot = [0, 0, 1]

        def qkv_fillers(c):
            # Q^T/K^T/V projections for chunk c, as 7 PE filler groups
            qs = c * QC
            qsl = slice(qs, qs + QC)

            def g_kk():
                ps_kk = fl_pool.tile([P, QC], f32, tag="fl", name="ps_kk")
                for ct in range(CT):
                    nc.tensor.matmul(ps_kk, lhsT=wqk_s[:, ct, 0, :],
                                     rhs=xT_s[:, ct, qsl],
                                     start=(ct == 0), stop=(ct == CT - 1))
                nc.vector.tensor_copy(out=kt_s[:, 0, qsl], in_=ps_kk)

            def g_qq():
                ps_qq = fl_pool.tile([P, QC], f32, tag="fl", name="ps_qq")
                for ct in range(CT):
                    nc.tensor.matmul(ps_qq, lhsT=wqk_s[:, ct, 1, :],
                                     rhs=xT_s[:, ct, qsl],
                                     start=(ct == 0), stop=(ct == CT - 1))
                nc.vector.tensor_scalar_add(out=qt_s[:, 0, qsl], in0=ps_qq,
                                            scalar1=bq_s[:, 0:1])

            def g_kq2():
                ps_kq2 = fl_pool.tile([P, QC], f32, tag="fl", name="ps_kq2")
                for ct in range(CT):
                    nc.tensor.matmul(ps_kq2, lhsT=wqk_s[:, ct, 2, :],
                                     rhs=xT_s[:, ct, qsl],
                                     start=(ct == 0), stop=(ct == CT - 1))
                nc.vector.tensor_copy(out=kt_s[0:HD, 1, qsl], in_=ps_kq2[0:HD, :])
                # head2 Q lands in parts 64:128; bias-add, then repartition DMA
                q2st = work.tile([P, QC], bf, tag="q2st", name="q2st")
                nc.vector.tensor_scalar_add(out=q2st[HD:P, :], in0=ps_kq2[HD:P, :],
                                            scalar1=bq_s[HD:P, 1:2])
                nc.sync.dma_start(out=qt_s[0:HD, 1, qsl], in_=q2st[HD:P, :])

            def g_v(kt):
                def f():
                    ps_v = fl_pool.tile([P, HPC * HD], f32, tag="fl", name="ps_v")
                    for ct in range(CT):
                        nc.tensor.matmul(ps_v,
                                         lhsT=xT_s[:, ct, kt * P:(kt + 1) * P],
                                         rhs=wv_s[:, ct, :],
                                         start=(ct == 0), stop=(ct == CT - 1))
                    nc.vector.tensor_copy(
                        out=v_s[:, kt, :, 0:HD],
                        in_=ps_v.rearrange("p (h d) -> p h d", h=HPC))
                return f
            return [g_kk, g_qq, g_kq2] + [g_v(kt) for kt in (2 * c, 2 * c + 1)]

        def proj_fillers(t, last=False):
            # output projection of q-tile t, as 2 column-group pieces that
            # share one bf16 staging tile; the second piece sends the DMA.
            # The final q-tile instead DMAs each piece straight from PSUM
            # (fp32, two queues) to cut the tail latency.
            holder = {}

            def piece(e0, en):
                def f():
                    pp = fl_pool.tile([P, en], f32, tag="fl", name="pp")
                    nc.tensor.matmul(pp, lhsT=attn01T[:, t * P:(t + 1) * P],
                                     rhs=wpa_s[:, e0:e0 + en],
                                     start=True, stop=False)
                    nc.tensor.matmul(pp, lhsT=attn2T[0:HD, t * P:(t + 1) * P],
                                     rhs=wpb_s[:, e0:e0 + en],
                                     start=False, stop=True)
                    if last:
                        # split the final tile across DVE+ACT copies and two
                        # DMA queues so the tail isn't one long serial chain
                        if e0 == 0:
                            holder["ob"] = outs_pool.tile([P, D], bf, tag="ob",
                                                          name="ob")
                            nc.vector.tensor_copy(out=holder["ob"][:, 0:en],
                                                  in_=pp)
                            nc.sync.dma_start(out=out_p[t * P:(t + 1) * P, 0:en],
                                              in_=holder["ob"][:, 0:en])
                        else:
                            nc.scalar.copy(out=holder["ob"][:, e0:e0 + en],
                                           in_=pp)
                            nc.scalar.dma_start(
                                out=out_p[t * P:(t + 1) * P, e0:e0 + en],
                                in_=holder["ob"][:, e0:e0 + en])
                        return
                    if e0 == 0:
                        holder["ob"] = outs_pool.tile([P, D], bf, tag="ob",
                                                      name="ob")
                    ob = holder["ob"]
                    nc.vector.tensor_copy(out=ob[:, e0:e0 + en], in_=pp)
                    if e0 != 0:
                        nc.sync.dma_start(out=out_p[t * P:(t + 1) * P, :], in_=ob)
                return f
            return [piece(0, 512), piece(512, 256)]

        def emit_scores(kt, c, ss_t):
            qs = c * QC
            off = P if kt == 2 * c + 1 else 0
            n = QC - off
            for h in range(HPC):
                nc.tensor.matmul(ss_t[:, h, 0:n],
                                 lhsT=kt_s[hsl[h], hslot[h], kt * P:(kt + 1) * P],
                                 rhs=qt_s[hsl[h], hslot[h], qs + off:qs + QC],
                                 start=True, stop=True)

        def emit_exp_mask(kt, c, ss_t, pt):
            off = P if kt == 2 * c + 1 else 0
            n = QC - off
            nc.scalar.activation(out=pt[:, kt, :, off:QC], in_=ss_t[:, :, 0:n],
                                 func=Exp, scale=0.125)
            if kt >= 2 * c:  # diagonal block: mask k>q inside the 128x128 square
                for h in range(HPC):
                    nc.gpsimd.tensor_mul(out=pt[:, kt, h, off:off + P],
                                         in0=pt[:, kt, h, off:off + P], in1=mask_s)

        def emit_av_tile(t, c, pt, pe_transpose=False):
            # AV for q-tile t (flipped: out [q, d+1]), one head at a time,
            # then fused normalize-divide into the ao staging slot, then
            # repartition via DMA xbar transpose (PE transpose in the tail).
            qi = t - 2 * c
            slot = t % 4
            for h in range(HPC):
                po = av_pool.tile([P, HD + 1], f32, tag="av", name="po")
                nkt = 2 * c + qi + 1
                for kt in range(nkt):
                    nc.tensor.matmul(po, lhsT=pt[:, kt, h, qi * P:(qi + 1) * P],
                                     rhs=v_s[:, kt, h, :],
                                     start=(kt == 0), stop=(kt == nkt - 1))
                nc.vector.tensor_scalar(out=ao_s[:, slot, h * HD:(h + 1) * HD],
                                        in0=po[:, 0:HD],
                                        scalar1=po[:, HD:HD + 1], scalar2=None,
                                        op0=mybir.AluOpType.divide)
            tsl = slice(t * P, (t + 1) * P)
            if pe_transpose:
                for half in range(2):
                    tp = fl_pool.tile([P, P], bf, tag="fl", name="tp")
                    nc.tensor.transpose(tp, ao_s[:, slot, half * P:(half + 1) * P],
                                        ident_s)
                    dst = attn01T if half == 0 else attn2T
                    nc.vector.tensor_copy(out=dst[:, tsl], in_=tp)
            else:
                nc.sync.dma_start_transpose(attn01T[:, tsl], ao_s[:, slot, 0:P])
                nc.sync.dma_start_transpose(attn2T[:, tsl], ao_s[:, slot, P:2 * P])

        for f in qkv_fillers(0):
            f()

        # Projection tiles are deferred toward the late, exp-heavy chunks
        # where the PE would otherwise starve waiting on ACT. proj(t) may run
        # any chunk after t's transposes (end of chunk t//2); quotas sized to
        # each chunk's PE-vs-ACT deficit.
        proj_quota = {4: 1, 5: 2, 6: 3, 7: 8}
        proj_next = 0  # next q-tile whose projection is still unscheduled
        pending_av = None  # deferred odd-q-tile AV from the previous chunk

        for c in range(NQC):
            nkt = 2 * c + 2
            pt = pt_s[c % 2]
            last = c == NQC - 1
            # fillers woven into this chunk's attention: next chunk's QKV,
            # then deferred projections (ready through q-tile 2c-1)
            fillers = []
            if not last:
                fillers += qkv_fillers(c + 1)
            # q-tiles with transposes complete before chunk c (the odd tile
            # of chunk c-1 is deferred into this chunk, so exclude it)
            ready = max(0, 2 * c - 1)
            for _ in range(proj_quota.get(c, 0)):
                if proj_next < ready:
                    fillers += proj_fillers(proj_next)
                    proj_next += 1
            emitted = 0
            n_fill = len(fillers)

            prev = None
            for kt in range(nkt):
                ss_t = ss_pool.tile([P, HPC, QC], f32, tag="ss", name="ss_t")
                emit_scores(kt, c, ss_t)
                if kt == 0 and pending_av is not None:
                    # previous chunk's odd q-tile: emitted here so the PE can
                    # run ahead into this chunk while its last exp finishes
                    pending_av()
                if prev is not None:
                    emit_exp_mask(kt - 1, c, prev, pt)
                prev = ss_t
                want = ((kt + 1) * n_fill) // (nkt + 1)
                while emitted < want:
                    fillers[emitted]()
                    emitted += 1
                if kt == nkt - 1:
                    # q-tile 2c only needs key blocks <= 2c: runs during the
                    # last key block's scores/exp
                    emit_exp_mask(kt, c, prev, pt)
                    prev = None
                    emit_av_tile(2 * c, c, pt, pe_transpose=last)
            while emitted < n_fill:
                fillers[emitted]()
                emitted += 1
            if last:
                emit_av_tile(2 * c + 1, c, pt, pe_transpose=True)
            else:
                pending_av = (lambda cc, pp: lambda: emit_av_tile(
                    2 * cc + 1, cc, pp))(c, pt)

        # tail: any remaining projections (final two q-tiles at least)
        for t in range(proj_next, 2 * NQC):
            for f in proj_fillers(t, last=(t == 2 * NQC - 1)):
                f()

    nc.compile()
    return nc


def _prep_inputs(x, W_qkv, b_qkv, W_proj):
    """Build the 8 per-core input maps (all bf16 except biases)."""
    in_maps = []
    for cid in range(NCORES):
        b, g = divmod(cid, 4)
        hs = [g * HPC + i for i in range(HPC)]  # global head ids

        def wslice(kind, h):  # kind 0=q 1=k 2=v
            return W_qkv[:, kind * D + h * HD:(kind * D + (h + 1) * HD)]

        xT = np.ascontiguousarray(x[b].T).astype(_BF)

        w_qk = np.zeros((D, 3, P), dtype=np.float32)
        w_qk[:, 0, 0:HD] = wslice(1, hs[0])
        w_qk[:, 0, HD:P] = wslice(1, hs[1])
        w_qk[:, 1, 0:HD] = wslice(0, hs[0])
        w_qk[:, 1, HD:P] = wslice(0, hs[1])
        w_qk[:, 2, 0:HD] = wslice(1, hs[2])
        w_qk[:, 2, HD:P] = wslice(0, hs[2])

        w_v = np.concatenate([wslice(2, h) for h in hs], axis=1)

        bq = np.zeros((P, 2), dtype=np.float32)
        bq[0:HD, 0] = b_qkv[hs[0] * HD:(hs[0] + 1) * HD]
        bq[HD:P, 0] = b_qkv[hs[1] * HD:(hs[1] + 1) * HD]
        bq[HD:P, 1] = b_qkv[hs[2] * HD:(hs[2] + 1) * HD]

        w_p = np.concatenate([W_proj[h * HD:(h + 1) * HD, :] for h in hs], axis=0)

        mask = np.triu(np.ones((P, P), dtype=np.float32))

        in_maps.append({
            "xT": xT,
            "w_qk": w_qk.astype(_BF),
            "w_v": w_v.astype(_BF),
            "bq": bq,
            "w_p": w_p.astype(_BF),
            "mask": mask.astype(_BF),
            "ident": np.eye(P, dtype=np.float32).astype(_BF),
        })
    return in_maps


def _run(inputs, trace=False):
    from concourse.bass_utils import run_bass_kernel_spmd

    x = np.asarray(inputs["x"], dtype=np.float32)
    W_qkv = np.asarray(inputs["W_qkv"], dtype=np.float32)
    b_qkv = np.asarray(inputs["b_qkv"], dtype=np.float32)
    W_proj = np.asarray(inputs["W_proj"], dtype=np.float32)
    b_proj = np.asarray(inputs["b_proj"], dtype=np.float32)

    if "nc" not in _cache:
        _cache["nc"] = _build_nc()
    nc = _cache["nc"]

    in_maps = _prep_inputs(x, W_qkv, b_qkv, W_proj)
    res = run_bass_kernel_spmd(nc, in_maps, core_ids=list(range(NCORES)),
                               trace=trace)

    host_bias = b_proj + b_qkv[2 * D:3 * D] @ W_proj  # b_v folded through proj
    B = x.shape[0]
    out = np.zeros((B, S, D), dtype=np.float32)
    for cid in range(NCORES):
        b = cid // 4
        out[b] += res.results[cid]["out_p"].astype(np.float32)
    out += host_bias
    return out, res


def kernel(x, W_qkv, b_qkv, W_proj, b_proj):
    out, _ = _run({"x": x, "W_qkv": W_qkv, "b_qkv": b_qkv,
                   "W_proj": W_proj, "b_proj": b_proj})
    return out
